# revision 31
# baseline (speedup 1.0000x reference)
"""Trainium2 Bass kernel for nn_AdaptiveFusion.

Math (per batch b):
  q  = x @ Wq.T + bq                         (L,H)
  kv = g @ Wkv.T + bkv ; k,v = split         (Lg,H) each
  p  = softmax(q @ k.T / sqrt(H))            (L,Lg)
  gc = p @ v                                 (L,H)
  g1 = sigmoid(x @ W1x.T + gc @ W1y.T + bg1) (L,H)   [k-independent]
  h1 = gc + g1*(x - gc)                      (L,H)
  A  = h1 @ W2x.T                            (L,H)
  C  = s @ W2y.T + bg2                       (K,H)
  out[l,k,o] = s[k,o] + sigmoid(A[l,o]+C[k,o]) * (h1[l,o]-s[k,o])

Sharding: data-parallel over B (8 batches -> 8 cores), weights replicated,
no collectives.

Output stage (L*K*H = 6.3M elems/core), balanced across ACT and DVE:
  k <  KF : sig = Sigmoid(A + C_k) per-k on ACT (bias trick, PSUM input)
  k >= KF : arg = A + C_rep via one DVE TT (2x), then one batched ACT sigmoid
  combine : d = h1 - s_rep ; m = d*sig ; out = m + s_rep -- all big
            tensor_tensor at the 2x bf16 perf mode. s is pre-replicated
            host-side along a 64-wide dummy-l axis (layout prep only) so
            no DVE operand has a stride-0 inner dim.
Output DMA is written in [OC, 128, K, L] layout (exactly the SBUF tile
layout -> fully linear descriptors); the host permutes back.
"""

import os
import sys

import numpy as np

if "/opt/trn_rl_repo" not in sys.path:
    sys.path.insert(0, "/opt/trn_rl_repo")

import ml_dtypes

BF16 = ml_dtypes.bfloat16

B, L, K, Lg, H = 8, 256, 32, 128, 768
HC = H // 128  # h-chunks
OC = H // 128  # o-chunks
LR = 64        # dummy-l width of the host-replicated s
KF = 22        # k's handled by fused per-k ACT sigmoid; rest via DVE arg
KS = K - KF

_CACHE = {}

last_exec_time_ns = None
last_profile = None


def _build():
    import concourse.bacc as bacc
    import concourse.bass as bass
    import concourse.mybir as mybir
    import concourse.tile as tile

    f32 = mybir.dt.float32
    bf16 = mybir.dt.bfloat16
    AF = mybir.ActivationFunctionType
    OP = mybir.AluOpType

    nc = bacc.Bacc(None, target_bir_lowering=False, debug=False)

    # ---- DRAM parameters (per-core shard), all host-pre-swizzled to the
    # exact SBUF layout so every input DMA is fully linear.  Small tensors
    # are byte-packed into one "acts" param: per-partition DMA packets are
    # otherwise too small to get queue throughput. ----
    u8 = mybir.dt.uint8
    # acts layout per partition: bstack f32 (120B) | xT (3072B) | gT (1536B)
    #                            | sT (384B) | ident (256B)
    ABYTES = 120 + 3072 + 1536 + 384 + 256
    acts = nc.declare_dram_parameter("acts", [128, ABYTES], u8, isOutput=False)
    srep = nc.declare_dram_parameter("srep", [128, OC, K, LR], bf16, isOutput=False)
    wq_d = nc.declare_dram_parameter("wq", [128, HC, H], bf16, isOutput=False)
    wk_d = nc.declare_dram_parameter("wk", [128, HC, H], bf16, isOutput=False)
    wv_d = nc.declare_dram_parameter("wv", [128, HC, H], bf16, isOutput=False)
    w1x_d = nc.declare_dram_parameter("w1x", [128, HC, H], bf16, isOutput=False)
    w1y_d = nc.declare_dram_parameter("w1y", [128, HC, H], bf16, isOutput=False)
    w2x_d = nc.declare_dram_parameter("w2x", [128, HC, H], bf16, isOutput=False)
    w2y_d = nc.declare_dram_parameter("w2y", [128, HC, H], bf16, isOutput=False)
    out_d = nc.declare_dram_parameter("out", [OC, 128, K, L], bf16, isOutput=True)

    inv_sqrt_h = 1.0 / float(np.sqrt(H))

    def wload(e0, e1, name, src):
        # each weight split across two queues so it lands in half the time
        t = wpool_ref[0].tile([128, HC, H], bf16, tag=name)
        e0.dma_start(t[:, 0:HC // 2], src[:, 0:HC // 2])
        e1.dma_start(t[:, HC // 2:], src[:, HC // 2:])
        return t

    with tile.TileContext(nc) as tc:
        with (
            tc.tile_pool(name="wpool", bufs=1) as wpool,
            tc.tile_pool(name="apool", bufs=1) as apool,
            tc.tile_pool(name="ppool", bufs=3, space=bass.MemorySpace.PSUM) as ppool,
            tc.tile_pool(name="vpool", bufs=1, space=bass.MemorySpace.PSUM) as vpool,
            tc.tile_pool(name="atp", bufs=1, space=bass.MemorySpace.PSUM) as atp,
            tc.tile_pool(name="spool", bufs=3) as spool,
            tc.tile_pool(name="opool", bufs=2) as opool,
            tc.tile_pool(name="dpool", bufs=1) as dpool,
        ):
            wpool_ref = [wpool]
            # ---- DMA queues (all linear transfers), ordered by first use ----
            # scalar: packed small tensors first, then w2y
            acts_s = apool.tile([128, ABYTES], u8)
            nc.scalar.dma_start(acts_s[:], acts[:])
            bst_s = acts_s[:, 0:120].bitcast(f32)
            xT_s = acts_s[:, 120:3192].bitcast(bf16).rearrange(
                "p (c l) -> p c l", c=HC)
            gT_s = acts_s[:, 3192:4728].bitcast(bf16).rearrange(
                "p (c l) -> p c l", c=HC)
            sT_s = acts_s[:, 4728:5112].bitcast(bf16).rearrange(
                "p (c l) -> p c l", c=HC)
            id_s = acts_s[:, 5112:5368].bitcast(bf16)
            w2y_s = wload(nc.scalar, nc.scalar, "w2y", w2y_d)
            # sync + gpsimd: weight halves in use order, then srep halves
            wk_s = wload(nc.sync, nc.gpsimd, "wk", wk_d)
            wq_s = wload(nc.sync, nc.gpsimd, "wq", wq_d)
            wv_s = wload(nc.sync, nc.gpsimd, "wv", wv_d)
            srep_s = apool.tile([128, OC, K, LR], bf16)
            nc.sync.dma_start(srep_s[:, 0:OC // 2], srep[:, 0:OC // 2])
            nc.gpsimd.dma_start(srep_s[:, OC // 2:], srep[:, OC // 2:])
            w1x_s = wload(nc.sync, nc.gpsimd, "w1x", w1x_d)
            w1y_s = wload(nc.sync, nc.gpsimd, "w1y", w1y_d)
            w2x_s = wload(nc.sync, nc.gpsimd, "w2x", w2x_d)

            bqs_s, bk_s, bv_s = bst_s[:, 0:OC], bst_s[:, OC:2 * OC], bst_s[:, 2 * OC:3 * OC]
            bg1_s, bg2_s = bst_s[:, 3 * OC:4 * OC], bst_s[:, 4 * OC:5 * OC]

            # warm the ACT tables (Exp + Sigmoid) while DMA streams in
            scratch = spool.tile([128, 2], f32, tag="warm")
            nc.scalar.activation(scratch[:, 0:1], bst_s[:, 0:1], AF.Exp)
            nc.scalar.activation(scratch[:, 1:2], bst_s[:, 0:1], AF.Sigmoid)

            # ---- k^T[o,m] ----
            kT_s = apool.tile([128, OC, Lg], bf16)
            for oc in range(OC):
                ps = ppool.tile([128, Lg], f32, tag="ps")
                for hc in range(HC):
                    nc.tensor.matmul(
                        ps[:], wk_s[:, hc, oc * 128:(oc + 1) * 128], gT_s[:, hc, :],
                        start=(hc == 0), stop=(hc == HC - 1))
                nc.scalar.activation(kT_s[:, oc, :], ps[:], AF.Identity,
                                     bias=bk_s[:, oc:oc + 1])

            # ---- q^T[o,l] ----
            qT_s = apool.tile([128, OC, L], bf16)
            for oc in range(OC):
                ps = ppool.tile([128, L], f32, tag="ps")
                for hc in range(HC):
                    nc.tensor.matmul(
                        ps[:], wq_s[:, hc, oc * 128:(oc + 1) * 128], xT_s[:, hc, :],
                        start=(hc == 0), stop=(hc == HC - 1))
                nc.scalar.activation(qT_s[:, oc, :], ps[:], AF.Identity,
                                     bias=bqs_s[:, oc:oc + 1], scale=inv_sqrt_h)

            # ---- v[m,o] (no bias; bv folded into gctx copy) ----
            v_s = apool.tile([128, H], bf16)
            psv = vpool.tile([128, H], f32, tag="psv")
            for third in range(3):
                sl = slice(third * 256, (third + 1) * 256)
                for hc in range(HC):
                    nc.tensor.matmul(psv[:, sl], gT_s[:, hc, :], wv_s[:, hc, sl],
                                     start=(hc == 0), stop=(hc == HC - 1))
            nc.scalar.activation(v_s[:], psv[:], AF.Copy)

            # ---- scores + softmax + transpose(probs) ----
            probsT_s = apool.tile([128, 2, 128], bf16)  # [m, lb, l]
            for lb in range(2):
                pss = ppool.tile([128, Lg], f32, tag="ps")
                for oc in range(OC):
                    nc.tensor.matmul(
                        pss[:], qT_s[:, oc, lb * 128:(lb + 1) * 128], kT_s[:, oc, :],
                        start=(oc == 0), stop=(oc == OC - 1))
                nmax = spool.tile([128, 1], f32, tag="nmax")
                nc.vector.tensor_reduce(nmax[:], pss[:], mybir.AxisListType.X,
                                        OP.max, negate=True)
                e_s = spool.tile([128, Lg], bf16, tag="es")
                ssum = spool.tile([128, 1], f32, tag="ssum")
                nc.scalar.activation(e_s[:], pss[:], AF.Exp,
                                     bias=nmax[:], accum_out=ssum[:])
                rcp = spool.tile([128, 1], f32, tag="rcp")
                nc.vector.reciprocal(rcp[:], ssum[:])
                pr_s = spool.tile([128, Lg], bf16, tag="prs")
                nc.vector.tensor_scalar_mul(pr_s[:], e_s[:], rcp[:])
                pst = ppool.tile([128, 128], bf16, tag="ps")
                nc.tensor.transpose(pst[:], pr_s[:], id_s[:])
                nc.scalar.activation(probsT_s[:, lb, :], pst[:], AF.Copy)

            # ---- C^T[o,k]+bg2 (first: small, weight arrives first) ----
            cb_s = apool.tile([128, OC, K], f32)
            for oc in range(OC):
                psc = ppool.tile([128, K], f32, tag="ps")
                for hc in range(HC):
                    nc.tensor.matmul(
                        psc[:], w2y_s[:, hc, oc * 128:(oc + 1) * 128], sT_s[:, hc, :],
                        start=(hc == 0), stop=(hc == HC - 1))
                nc.scalar.activation(cb_s[:, oc, :], psc[:], AF.Identity,
                                     bias=bg2_s[:, oc:oc + 1])

            # ---- gctx^T[o,l] (bv folded in via bias) ----
            gcT_s = apool.tile([128, OC, L], bf16)
            for oc in range(OC):
                psg = ppool.tile([128, L], f32, tag="ps")
                for lb in range(2):
                    nc.tensor.matmul(
                        psg[:, lb * 128:(lb + 1) * 128],
                        v_s[:, oc * 128:(oc + 1) * 128], probsT_s[:, lb, :],
                        start=True, stop=True)
                nc.scalar.activation(gcT_s[:, oc, :], psg[:], AF.Identity,
                                     bias=bv_s[:, oc:oc + 1])

            # ---- gate1 + h1^T ----
            h1_s = apool.tile([128, HC, L], bf16)
            for oc in range(OC):
                ps1 = ppool.tile([128, L], f32, tag="ps")
                for hc in range(HC):
                    nc.tensor.matmul(
                        ps1[:], w1x_s[:, hc, oc * 128:(oc + 1) * 128], xT_s[:, hc, :],
                        start=(hc == 0), stop=False)
                for hc in range(HC):
                    nc.tensor.matmul(
                        ps1[:], w1y_s[:, hc, oc * 128:(oc + 1) * 128], gcT_s[:, hc, :],
                        start=False, stop=(hc == HC - 1))
                g1_s = spool.tile([128, L], bf16, tag="g1")
                nc.scalar.activation(g1_s[:], ps1[:], AF.Sigmoid,
                                     bias=bg1_s[:, oc:oc + 1])
                d1 = spool.tile([128, L], bf16, tag="d1")
                nc.vector.tensor_sub(d1[:], xT_s[:, oc, :], gcT_s[:, oc, :])
                m1 = spool.tile([128, L], bf16, tag="m1")
                nc.vector.tensor_mul(m1[:], d1[:], g1_s[:])
                nc.vector.tensor_add(h1_s[:, oc, :], m1[:], gcT_s[:, oc, :])

            # ---- A = h1 @ W2x per oc, then the output pipeline ----
            A_sb = apool.tile([128, OC, L], bf16)
            at_p0 = atp.tile([128, 2, L], f32, tag="at0")
            at_p1 = atp.tile([128, 2, L], f32, tag="at1")
            at_p2 = atp.tile([128, 2, L], f32, tag="at2")
            at_tiles = [at_p0, at_p1, at_p2]
            NLC = L // LR

            qrot = [nc.sync, nc.gpsimd]
            for oc in range(OC):
                atv = at_tiles[oc // 2][:, oc % 2, :]
                for hc in range(HC):
                    nc.tensor.matmul(
                        atv, w2x_s[:, hc, oc * 128:(oc + 1) * 128],
                        h1_s[:, hc, :], start=(hc == 0), stop=(hc == HC - 1))
                # A to SBUF once: sigmoids avoid the extra PSUM-access bubble
                nc.scalar.activation(A_sb[:, oc, :], atv, AF.Copy)

                # d = h1 - s: single TT per oc; srep reused via a stride-0
                # middle dim so the packed inner dim keeps the 2x mode
                dbuf = dpool.tile([128, K, L], bf16, tag="dbuf")
                nc.vector.tensor_sub(
                    dbuf[:].rearrange("p k (a b) -> p k a b", a=NLC),
                    h1_s[:, oc, :].rearrange("p (a b) -> p a b", a=NLC)
                    .unsqueeze(1).broadcast_to([128, K, NLC, LR]),
                    srep_s[:, oc].unsqueeze(2).broadcast_to([128, K, NLC, LR]))

                # sigmoids: k < KF fused on ACT; k >= KF via DVE arg + one ACT
                sig = opool.tile([128, K, L], bf16, tag="sig")
                for k in range(KF):
                    nc.scalar.activation(sig[:, k, :], A_sb[:, oc, :], AF.Sigmoid,
                                         bias=cb_s[:, oc, k:k + 1])
                if KS:
                    crep = spool.tile([128, KS, LR], bf16, tag="crep")
                    nc.vector.tensor_copy(
                        crep[:], cb_s[:, oc, KF:].unsqueeze(2)
                        .broadcast_to([128, KS, LR]))
                    arg = dpool.tile([128, KS, L], bf16, tag="arg")
                    nc.vector.tensor_add(
                        arg[:].rearrange("p k (a b) -> p k a b", a=NLC),
                        A_sb[:, oc, :].rearrange("p (a b) -> p a b", a=NLC)
                        .unsqueeze(1).broadcast_to([128, KS, NLC, LR]),
                        crep[:].unsqueeze(2).broadcast_to([128, KS, NLC, LR]))
                    nc.scalar.activation(
                        sig[:, KF:, :].rearrange("p k l -> p (k l)"),
                        arg[:].rearrange("p k l -> p (k l)"), AF.Sigmoid)

                # m = d * sig in k-halves; out = m + s; DMA per k-half so all
                # three queues stream output continuously
                ob = opool.tile([128, K, L], bf16, tag="ob")
                KH = K // 2
                for kh in range(2):
                    ksl = slice(kh * KH, (kh + 1) * KH)
                    nc.vector.tensor_mul(
                        ob[:, ksl, :].rearrange("p k l -> p (k l)"),
                        dbuf[:, ksl, :].rearrange("p k l -> p (k l)"),
                        sig[:, ksl, :].rearrange("p k l -> p (k l)"))
                    nc.vector.tensor_add(
                        ob[:, ksl, :].rearrange("p k (a b) -> p k a b", a=NLC),
                        ob[:, ksl, :].rearrange("p k (a b) -> p k a b", a=NLC),
                        srep_s[:, oc, ksl].unsqueeze(2)
                        .broadcast_to([128, KH, NLC, LR]))
                    if oc == OC - 1:
                        KQ = KH // 2
                        for kq in range(2):
                            qs = slice(kh * KH + kq * KQ, kh * KH + (kq + 1) * KQ)
                            qrot[(2 * kh + kq) % 2].dma_start(
                                out_d[oc][:, qs], ob[:, qs, :])
                    else:
                        qrot[(2 * oc + kh) % 2].dma_start(
                            out_d[oc][:, ksl], ob[:, ksl, :])

    nc.compile()
    return nc


def _prep_in_maps(x, s, g, Wq, bq, Wkv, bkv, Wg1, bg1, Wg2, bg2):
    def swz(a):  # [X, H-contract] -> [128, HC, X]: SBUF layout, h on partitions
        aT = np.asarray(a).T  # [H, X]
        return np.ascontiguousarray(
            aT.reshape(HC, 128, -1).transpose(1, 0, 2)).astype(BF16)

    def rsh(v):  # (H,) -> [128, H//128] partition-major chunks
        return np.ascontiguousarray(v.reshape(OC, 128).T).astype(np.float32)

    Wk, Wv = Wkv[:H], Wkv[H:]
    W1x, W1y = Wg1[:, :H], Wg1[:, H:]
    W2x, W2y = Wg2[:, :H], Wg2[:, H:]
    shared = {
        "wq": swz(Wq), "wk": swz(Wk), "wv": swz(Wv),
        "w1x": swz(W1x), "w1y": swz(W1y), "w2x": swz(W2x),
        "w2y": swz(W2y),
    }
    bstack = np.concatenate(
        [rsh(bq / np.sqrt(H)), rsh(bkv[:H]), rsh(bkv[H:]), rsh(bg1), rsh(bg2)],
        axis=1)
    ident = np.eye(128, dtype=np.float32).astype(BF16)
    in_maps = []
    for b in range(B):
        m = dict(shared)
        # byte-packed small tensors: bstack | xT | gT | sT | ident
        m["acts"] = np.concatenate(
            [bstack.view(np.uint8).reshape(128, -1),
             swz(x[b]).view(np.uint8).reshape(128, -1),
             swz(g[b]).view(np.uint8).reshape(128, -1),
             swz(s[b]).view(np.uint8).reshape(128, -1),
             ident.view(np.uint8).reshape(128, -1)], axis=1)
        # [128, OC, K, LR]: s[k, oc*128+p] replicated along a dummy-l axis
        sr = np.asarray(s[b]).T.reshape(OC, 128, K).transpose(1, 0, 2)
        m["srep"] = np.ascontiguousarray(
            np.broadcast_to(sr[..., None], (128, OC, K, LR))).astype(BF16)
        in_maps.append(m)
    return in_maps


def kernel(**inputs):
    global last_exec_time_ns, last_profile
    from concourse.bass_utils import run_bass_kernel_spmd

    if "nc" not in _CACHE:
        _CACHE["nc"] = _build()
    nc = _CACHE["nc"]

    inputs = {k: np.asarray(v, dtype=np.float32) if np.asarray(v).dtype != np.int32
              else np.asarray(v) for k, v in inputs.items()}
    in_maps = _prep_in_maps(**inputs)

    trace = bool(int(os.environ.get("BASS_KERNEL_TRACE", "0")))
    repeat = int(os.environ.get("BASS_KERNEL_REPEAT", "1"))
    times = []
    for _ in range(repeat):
        res = run_bass_kernel_spmd(nc, in_maps, core_ids=list(range(B)), trace=trace)
        if res.exec_time_ns is not None:
            times.append(res.exec_time_ns)
    if times:
        print(f"exec times: {times}")
        last_exec_time_ns = min(times)
    last_profile = res.profile_json

    out = np.empty((B, L, K, H), dtype=np.float32)
    for b in range(B):
        # per-core result is [OC, 128, K, L] -> [L, K, H]
        r = res.results[b]["out"].astype(np.float32)
        out[b] = np.transpose(r, (3, 2, 0, 1)).reshape(L, K, H)
    return out


# revision 32
# speedup vs baseline: 1.0038x; 1.0038x over previous
"""Trainium2 Bass kernel for nn_AdaptiveFusion.

Math (per batch b):
  q  = x @ Wq.T + bq                         (L,H)
  kv = g @ Wkv.T + bkv ; k,v = split         (Lg,H) each
  p  = softmax(q @ k.T / sqrt(H))            (L,Lg)
  gc = p @ v                                 (L,H)
  g1 = sigmoid(x @ W1x.T + gc @ W1y.T + bg1) (L,H)   [k-independent]
  h1 = gc + g1*(x - gc)                      (L,H)
  A  = h1 @ W2x.T                            (L,H)
  C  = s @ W2y.T + bg2                       (K,H)
  out[l,k,o] = s[k,o] + sigmoid(A[l,o]+C[k,o]) * (h1[l,o]-s[k,o])

Sharding: data-parallel over B (8 batches -> 8 cores), weights replicated,
no collectives.

Output stage (L*K*H = 6.3M elems/core), balanced across ACT and DVE:
  k <  KF : sig = Sigmoid(A + C_k) per-k on ACT (bias trick, PSUM input)
  k >= KF : arg = A + C_rep via one DVE TT (2x), then one batched ACT sigmoid
  combine : d = h1 - s_rep ; m = d*sig ; out = m + s_rep -- all big
            tensor_tensor at the 2x bf16 perf mode. s is pre-replicated
            host-side along a 64-wide dummy-l axis (layout prep only) so
            no DVE operand has a stride-0 inner dim.
Output DMA is written in [OC, 128, K, L] layout (exactly the SBUF tile
layout -> fully linear descriptors); the host permutes back.
"""

import os
import sys

import numpy as np

if "/opt/trn_rl_repo" not in sys.path:
    sys.path.insert(0, "/opt/trn_rl_repo")

import ml_dtypes

BF16 = ml_dtypes.bfloat16

B, L, K, Lg, H = 8, 256, 32, 128, 768
HC = H // 128  # h-chunks
OC = H // 128  # o-chunks
LR = 16        # dummy-l width of the host-replicated s
KF = 22        # k's handled by fused per-k ACT sigmoid; rest via DVE arg
KS = K - KF

_CACHE = {}

last_exec_time_ns = None
last_profile = None


def _build():
    import concourse.bacc as bacc
    import concourse.bass as bass
    import concourse.mybir as mybir
    import concourse.tile as tile

    f32 = mybir.dt.float32
    bf16 = mybir.dt.bfloat16
    AF = mybir.ActivationFunctionType
    OP = mybir.AluOpType

    nc = bacc.Bacc(None, target_bir_lowering=False, debug=False)

    # ---- DRAM parameters (per-core shard), all host-pre-swizzled to the
    # exact SBUF layout so every input DMA is fully linear.  Small tensors
    # are byte-packed into one "acts" param: per-partition DMA packets are
    # otherwise too small to get queue throughput. ----
    u8 = mybir.dt.uint8
    # acts layout per partition: bstack f32 (120B) | xT (3072B) | gT (1536B)
    #                            | sT (384B) | ident (256B)
    ABYTES = 120 + 3072 + 1536 + 384 + 256
    acts = nc.declare_dram_parameter("acts", [128, ABYTES], u8, isOutput=False)
    srep = nc.declare_dram_parameter("srep", [128, OC, K, LR], bf16, isOutput=False)
    wq_d = nc.declare_dram_parameter("wq", [128, HC, H], bf16, isOutput=False)
    wk_d = nc.declare_dram_parameter("wk", [128, HC, H], bf16, isOutput=False)
    wv_d = nc.declare_dram_parameter("wv", [128, HC, H], bf16, isOutput=False)
    w1x_d = nc.declare_dram_parameter("w1x", [128, HC, H], bf16, isOutput=False)
    w1y_d = nc.declare_dram_parameter("w1y", [128, HC, H], bf16, isOutput=False)
    w2x_d = nc.declare_dram_parameter("w2x", [128, HC, H], bf16, isOutput=False)
    w2y_d = nc.declare_dram_parameter("w2y", [128, HC, H], bf16, isOutput=False)
    out_d = nc.declare_dram_parameter("out", [OC, 128, K, L], bf16, isOutput=True)

    inv_sqrt_h = 1.0 / float(np.sqrt(H))

    def wload(e0, e1, name, src):
        # each weight split across two queues so it lands in half the time
        t = wpool_ref[0].tile([128, HC, H], bf16, tag=name)
        e0.dma_start(t[:, 0:HC // 2], src[:, 0:HC // 2])
        e1.dma_start(t[:, HC // 2:], src[:, HC // 2:])
        return t

    with tile.TileContext(nc) as tc:
        with (
            tc.tile_pool(name="wpool", bufs=1) as wpool,
            tc.tile_pool(name="apool", bufs=1) as apool,
            tc.tile_pool(name="ppool", bufs=3, space=bass.MemorySpace.PSUM) as ppool,
            tc.tile_pool(name="vpool", bufs=1, space=bass.MemorySpace.PSUM) as vpool,
            tc.tile_pool(name="atp", bufs=1, space=bass.MemorySpace.PSUM) as atp,
            tc.tile_pool(name="spool", bufs=3) as spool,
            tc.tile_pool(name="opool", bufs=2) as opool,
            tc.tile_pool(name="dpool", bufs=1) as dpool,
        ):
            wpool_ref = [wpool]
            # ---- DMA queues (all linear transfers), ordered by first use ----
            # scalar: packed small tensors first, then w2y
            acts_s = apool.tile([128, ABYTES], u8)
            nc.scalar.dma_start(acts_s[:], acts[:])
            bst_s = acts_s[:, 0:120].bitcast(f32)
            xT_s = acts_s[:, 120:3192].bitcast(bf16).rearrange(
                "p (c l) -> p c l", c=HC)
            gT_s = acts_s[:, 3192:4728].bitcast(bf16).rearrange(
                "p (c l) -> p c l", c=HC)
            sT_s = acts_s[:, 4728:5112].bitcast(bf16).rearrange(
                "p (c l) -> p c l", c=HC)
            id_s = acts_s[:, 5112:5368].bitcast(bf16)
            srep_s = apool.tile([128, OC, K, LR], bf16)
            nc.scalar.dma_start(srep_s[:], srep[:])
            w2y_s = wload(nc.scalar, nc.scalar, "w2y", w2y_d)
            # sync + gpsimd: weight halves in use order
            wk_s = wload(nc.sync, nc.gpsimd, "wk", wk_d)
            wq_s = wload(nc.sync, nc.gpsimd, "wq", wq_d)
            wv_s = wload(nc.sync, nc.gpsimd, "wv", wv_d)
            w1x_s = wload(nc.sync, nc.gpsimd, "w1x", w1x_d)
            w1y_s = wload(nc.sync, nc.gpsimd, "w1y", w1y_d)
            w2x_s = wload(nc.sync, nc.gpsimd, "w2x", w2x_d)

            bqs_s, bk_s, bv_s = bst_s[:, 0:OC], bst_s[:, OC:2 * OC], bst_s[:, 2 * OC:3 * OC]
            bg1_s, bg2_s = bst_s[:, 3 * OC:4 * OC], bst_s[:, 4 * OC:5 * OC]

            # warm the ACT tables (Exp + Sigmoid) while DMA streams in
            scratch = spool.tile([128, 2], f32, tag="warm")
            nc.scalar.activation(scratch[:, 0:1], bst_s[:, 0:1], AF.Exp)
            nc.scalar.activation(scratch[:, 1:2], bst_s[:, 0:1], AF.Sigmoid)

            # ---- k^T[o,m] ----
            kT_s = apool.tile([128, OC, Lg], bf16)
            for oc in range(OC):
                ps = ppool.tile([128, Lg], f32, tag="ps")
                for hc in range(HC):
                    nc.tensor.matmul(
                        ps[:], wk_s[:, hc, oc * 128:(oc + 1) * 128], gT_s[:, hc, :],
                        start=(hc == 0), stop=(hc == HC - 1))
                nc.scalar.activation(kT_s[:, oc, :], ps[:], AF.Identity,
                                     bias=bk_s[:, oc:oc + 1])

            # ---- q^T[o,l] ----
            qT_s = apool.tile([128, OC, L], bf16)
            for oc in range(OC):
                ps = ppool.tile([128, L], f32, tag="ps")
                for hc in range(HC):
                    nc.tensor.matmul(
                        ps[:], wq_s[:, hc, oc * 128:(oc + 1) * 128], xT_s[:, hc, :],
                        start=(hc == 0), stop=(hc == HC - 1))
                nc.scalar.activation(qT_s[:, oc, :], ps[:], AF.Identity,
                                     bias=bqs_s[:, oc:oc + 1], scale=inv_sqrt_h)

            # ---- v[m,o] (no bias; bv folded into gctx copy) ----
            v_s = apool.tile([128, H], bf16)
            psv = vpool.tile([128, H], f32, tag="psv")
            for third in range(3):
                sl = slice(third * 256, (third + 1) * 256)
                for hc in range(HC):
                    nc.tensor.matmul(psv[:, sl], gT_s[:, hc, :], wv_s[:, hc, sl],
                                     start=(hc == 0), stop=(hc == HC - 1))
            nc.scalar.activation(v_s[:], psv[:], AF.Copy)

            # ---- scores + softmax + transpose(probs) ----
            probsT_s = apool.tile([128, 2, 128], bf16)  # [m, lb, l]
            for lb in range(2):
                pss = ppool.tile([128, Lg], f32, tag="ps")
                for oc in range(OC):
                    nc.tensor.matmul(
                        pss[:], qT_s[:, oc, lb * 128:(lb + 1) * 128], kT_s[:, oc, :],
                        start=(oc == 0), stop=(oc == OC - 1))
                nmax = spool.tile([128, 1], f32, tag="nmax")
                nc.vector.tensor_reduce(nmax[:], pss[:], mybir.AxisListType.X,
                                        OP.max, negate=True)
                e_s = spool.tile([128, Lg], bf16, tag="es")
                ssum = spool.tile([128, 1], f32, tag="ssum")
                nc.scalar.activation(e_s[:], pss[:], AF.Exp,
                                     bias=nmax[:], accum_out=ssum[:])
                rcp = spool.tile([128, 1], f32, tag="rcp")
                nc.vector.reciprocal(rcp[:], ssum[:])
                pr_s = spool.tile([128, Lg], bf16, tag="prs")
                nc.vector.tensor_scalar_mul(pr_s[:], e_s[:], rcp[:])
                pst = ppool.tile([128, 128], bf16, tag="ps")
                nc.tensor.transpose(pst[:], pr_s[:], id_s[:])
                nc.scalar.activation(probsT_s[:, lb, :], pst[:], AF.Copy)

            # ---- C^T[o,k]+bg2 (first: small, weight arrives first) ----
            cb_s = apool.tile([128, OC, K], f32)
            for oc in range(OC):
                psc = ppool.tile([128, K], f32, tag="ps")
                for hc in range(HC):
                    nc.tensor.matmul(
                        psc[:], w2y_s[:, hc, oc * 128:(oc + 1) * 128], sT_s[:, hc, :],
                        start=(hc == 0), stop=(hc == HC - 1))
                nc.scalar.activation(cb_s[:, oc, :], psc[:], AF.Identity,
                                     bias=bg2_s[:, oc:oc + 1])

            # ---- gctx^T[o,l] (bv folded in via bias) ----
            gcT_s = apool.tile([128, OC, L], bf16)
            for oc in range(OC):
                psg = ppool.tile([128, L], f32, tag="ps")
                for lb in range(2):
                    nc.tensor.matmul(
                        psg[:, lb * 128:(lb + 1) * 128],
                        v_s[:, oc * 128:(oc + 1) * 128], probsT_s[:, lb, :],
                        start=True, stop=True)
                nc.scalar.activation(gcT_s[:, oc, :], psg[:], AF.Identity,
                                     bias=bv_s[:, oc:oc + 1])

            # ---- gate1 + h1^T ----
            h1_s = apool.tile([128, HC, L], bf16)
            for oc in range(OC):
                ps1 = ppool.tile([128, L], f32, tag="ps")
                for hc in range(HC):
                    nc.tensor.matmul(
                        ps1[:], w1x_s[:, hc, oc * 128:(oc + 1) * 128], xT_s[:, hc, :],
                        start=(hc == 0), stop=False)
                for hc in range(HC):
                    nc.tensor.matmul(
                        ps1[:], w1y_s[:, hc, oc * 128:(oc + 1) * 128], gcT_s[:, hc, :],
                        start=False, stop=(hc == HC - 1))
                g1_s = spool.tile([128, L], bf16, tag="g1")
                nc.scalar.activation(g1_s[:], ps1[:], AF.Sigmoid,
                                     bias=bg1_s[:, oc:oc + 1])
                d1 = spool.tile([128, L], bf16, tag="d1")
                nc.vector.tensor_sub(d1[:], xT_s[:, oc, :], gcT_s[:, oc, :])
                m1 = spool.tile([128, L], bf16, tag="m1")
                nc.vector.tensor_mul(m1[:], d1[:], g1_s[:])
                nc.vector.tensor_add(h1_s[:, oc, :], m1[:], gcT_s[:, oc, :])

            # ---- A = h1 @ W2x per oc, then the output pipeline ----
            A_sb = apool.tile([128, OC, L], bf16)
            at_p0 = atp.tile([128, 2, L], f32, tag="at0")
            at_p1 = atp.tile([128, 2, L], f32, tag="at1")
            at_p2 = atp.tile([128, 2, L], f32, tag="at2")
            at_tiles = [at_p0, at_p1, at_p2]
            NLC = L // LR

            qrot = [nc.sync, nc.gpsimd]
            for oc in range(OC):
                atv = at_tiles[oc // 2][:, oc % 2, :]
                for hc in range(HC):
                    nc.tensor.matmul(
                        atv, w2x_s[:, hc, oc * 128:(oc + 1) * 128],
                        h1_s[:, hc, :], start=(hc == 0), stop=(hc == HC - 1))
                # A to SBUF once: sigmoids avoid the extra PSUM-access bubble
                nc.scalar.activation(A_sb[:, oc, :], atv, AF.Copy)

                # d = h1 - s: single TT per oc; srep reused via a stride-0
                # middle dim so the packed inner dim keeps the 2x mode
                dbuf = dpool.tile([128, K, L], bf16, tag="dbuf")
                nc.vector.tensor_sub(
                    dbuf[:].rearrange("p k (a b) -> p k a b", a=NLC),
                    h1_s[:, oc, :].rearrange("p (a b) -> p a b", a=NLC)
                    .unsqueeze(1).broadcast_to([128, K, NLC, LR]),
                    srep_s[:, oc].unsqueeze(2).broadcast_to([128, K, NLC, LR]))

                # sigmoids: k < KF fused on ACT; k >= KF via DVE arg + one ACT
                sig = opool.tile([128, K, L], bf16, tag="sig")
                for k in range(KF):
                    nc.scalar.activation(sig[:, k, :], A_sb[:, oc, :], AF.Sigmoid,
                                         bias=cb_s[:, oc, k:k + 1])
                if KS:
                    crep = spool.tile([128, KS, LR], bf16, tag="crep")
                    nc.vector.tensor_copy(
                        crep[:], cb_s[:, oc, KF:].unsqueeze(2)
                        .broadcast_to([128, KS, LR]))
                    arg = dpool.tile([128, KS, L], bf16, tag="arg")
                    nc.vector.tensor_add(
                        arg[:].rearrange("p k (a b) -> p k a b", a=NLC),
                        A_sb[:, oc, :].rearrange("p (a b) -> p a b", a=NLC)
                        .unsqueeze(1).broadcast_to([128, KS, NLC, LR]),
                        crep[:].unsqueeze(2).broadcast_to([128, KS, NLC, LR]))
                    nc.scalar.activation(
                        sig[:, KF:, :].rearrange("p k l -> p (k l)"),
                        arg[:].rearrange("p k l -> p (k l)"), AF.Sigmoid)

                # m = d * sig in k-halves; out = m + s; DMA per k-half so all
                # three queues stream output continuously
                ob = opool.tile([128, K, L], bf16, tag="ob")
                KH = K // 2
                for kh in range(2):
                    ksl = slice(kh * KH, (kh + 1) * KH)
                    nc.vector.tensor_mul(
                        ob[:, ksl, :].rearrange("p k l -> p (k l)"),
                        dbuf[:, ksl, :].rearrange("p k l -> p (k l)"),
                        sig[:, ksl, :].rearrange("p k l -> p (k l)"))
                    nc.vector.tensor_add(
                        ob[:, ksl, :].rearrange("p k (a b) -> p k a b", a=NLC),
                        ob[:, ksl, :].rearrange("p k (a b) -> p k a b", a=NLC),
                        srep_s[:, oc, ksl].unsqueeze(2)
                        .broadcast_to([128, KH, NLC, LR]))
                    if oc == OC - 1:
                        KQ = KH // 2
                        for kq in range(2):
                            qs = slice(kh * KH + kq * KQ, kh * KH + (kq + 1) * KQ)
                            qrot[(2 * kh + kq) % 2].dma_start(
                                out_d[oc][:, qs], ob[:, qs, :])
                    else:
                        qrot[(2 * oc + kh) % 2].dma_start(
                            out_d[oc][:, ksl], ob[:, ksl, :])

    nc.compile()
    return nc


def _prep_in_maps(x, s, g, Wq, bq, Wkv, bkv, Wg1, bg1, Wg2, bg2):
    def swz(a):  # [X, H-contract] -> [128, HC, X]: SBUF layout, h on partitions
        aT = np.asarray(a).T  # [H, X]
        return np.ascontiguousarray(
            aT.reshape(HC, 128, -1).transpose(1, 0, 2)).astype(BF16)

    def rsh(v):  # (H,) -> [128, H//128] partition-major chunks
        return np.ascontiguousarray(v.reshape(OC, 128).T).astype(np.float32)

    Wk, Wv = Wkv[:H], Wkv[H:]
    W1x, W1y = Wg1[:, :H], Wg1[:, H:]
    W2x, W2y = Wg2[:, :H], Wg2[:, H:]
    shared = {
        "wq": swz(Wq), "wk": swz(Wk), "wv": swz(Wv),
        "w1x": swz(W1x), "w1y": swz(W1y), "w2x": swz(W2x),
        "w2y": swz(W2y),
    }
    bstack = np.concatenate(
        [rsh(bq / np.sqrt(H)), rsh(bkv[:H]), rsh(bkv[H:]), rsh(bg1), rsh(bg2)],
        axis=1)
    ident = np.eye(128, dtype=np.float32).astype(BF16)
    in_maps = []
    for b in range(B):
        m = dict(shared)
        # byte-packed small tensors: bstack | xT | gT | sT | ident
        m["acts"] = np.concatenate(
            [bstack.view(np.uint8).reshape(128, -1),
             swz(x[b]).view(np.uint8).reshape(128, -1),
             swz(g[b]).view(np.uint8).reshape(128, -1),
             swz(s[b]).view(np.uint8).reshape(128, -1),
             ident.view(np.uint8).reshape(128, -1)], axis=1)
        # [128, OC, K, LR]: s[k, oc*128+p] replicated along a dummy-l axis
        sr = np.asarray(s[b]).T.reshape(OC, 128, K).transpose(1, 0, 2)
        m["srep"] = np.ascontiguousarray(
            np.broadcast_to(sr[..., None], (128, OC, K, LR))).astype(BF16)
        in_maps.append(m)
    return in_maps


def kernel(**inputs):
    global last_exec_time_ns, last_profile
    from concourse.bass_utils import run_bass_kernel_spmd

    if "nc" not in _CACHE:
        _CACHE["nc"] = _build()
    nc = _CACHE["nc"]

    inputs = {k: np.asarray(v, dtype=np.float32) if np.asarray(v).dtype != np.int32
              else np.asarray(v) for k, v in inputs.items()}
    in_maps = _prep_in_maps(**inputs)

    trace = bool(int(os.environ.get("BASS_KERNEL_TRACE", "0")))
    repeat = int(os.environ.get("BASS_KERNEL_REPEAT", "1"))
    times = []
    for _ in range(repeat):
        res = run_bass_kernel_spmd(nc, in_maps, core_ids=list(range(B)), trace=trace)
        if res.exec_time_ns is not None:
            times.append(res.exec_time_ns)
    if times:
        print(f"exec times: {times}")
        last_exec_time_ns = min(times)
    last_profile = res.profile_json

    out = np.empty((B, L, K, H), dtype=np.float32)
    for b in range(B):
        # per-core result is [OC, 128, K, L] -> [L, K, H]
        r = res.results[b]["out"].astype(np.float32)
        out[b] = np.transpose(r, (3, 2, 0, 1)).reshape(L, K, H)
    return out
